# revision 2
# baseline (speedup 1.0000x reference)
"""MoE layer (top-2 of 8 experts) on 8 Trainium2 NeuronCores, expert-parallel.

v2 vs baseline:
- Routing distributed: each core routes only its 1024-token slice (8 tiles),
  computes combine-weights for all 8 experts, and an on-device AllToAll
  exchanges them so core e ends up with cw[:, e] for all 8192 tokens.
- FFN in bf16 (x pre-cast on host, weights pre-cast bf16 on host): bf16
  transposes (1 cyc/row vs 2), halved DMA traffic, and weights streamed
  exactly once per run (baseline re-streamed all 50MB fp32 5x).
- Capacity 2560 -> 2304 compact slots via an [8 x 1024]-bucket compaction
  (K=288 vs actual seed max 282); FFN runs 2 f-half passes so hT fits SBUF.
- w2 contraction accumulated in PSUM (16 matmuls/bank) instead of DVE adds.

Routing numerics (fp32 transposes + split-K fp32r gate matmuls + sigmoid
renorm) are bit-identical to the baseline: min top2/top3 logit gap is 2.8e-6
so the top-2 selection must match the reference exactly.

Self-contained: hardcodes shapes x[4,2048,1024], 8 experts, H=1024, F=4096.
"""

import os

os.environ.setdefault("JAX_PLATFORMS", "")

import numpy as np
import ml_dtypes

BF16 = ml_dtypes.bfloat16

T, H, F, E = 8192, 1024, 4096, 8
P = 128
NCORES = 8
K = 288                  # per-bucket slot capacity ([8, 1024] bucket layout)
C = 8 * K                # 2304 compact slots per expert
CT = C // P              # 18 slot tiles
SUBS = [512, 512, 512, 512, 256]
assert sum(SUBS) == C
NTLOC = (T // NCORES) // P   # 8 routing tiles per core
HC = H // P              # 8 h-blocks
FT = F // P              # 32 f-blocks
FTP = FT // 2            # 16 f-blocks per pass

_cache: dict = {}


def _build_nc():
    import concourse.mybir as mybir
    import concourse.tile as tile
    from concourse import bacc
    from concourse.bass import IndirectOffsetOnAxis
    from concourse.masks import make_identity

    dt = mybir.dt
    Alu = mybir.AluOpType
    Act = mybir.ActivationFunctionType

    nc = bacc.Bacc("TRN2", target_bir_lowering=False, num_devices=NCORES)

    xs_in = nc.dram_tensor("xs", [T // NCORES, H], dt.float32, kind="ExternalInput")
    xb_in = nc.dram_tensor("xb", [T, H], dt.bfloat16, kind="ExternalInput")
    gwt_in = nc.dram_tensor("gwt", [P, HC, E], dt.float32, kind="ExternalInput")
    w1_in = nc.dram_tensor("w1t", [FT, P, HC, P], dt.bfloat16, kind="ExternalInput")
    w3_in = nc.dram_tensor("w3t", [FT, P, HC, P], dt.bfloat16, kind="ExternalInput")
    w2_in = nc.dram_tensor("w2t", [FT, P, H], dt.bfloat16, kind="ExternalInput")

    y_out = nc.dram_tensor("y", [2, C, H], dt.float32, kind="ExternalOutput")
    idx_out = nc.dram_tensor("idx", [C], dt.int32, kind="ExternalOutput")

    cw_src = nc.dram_tensor("cw_src", [E, T // NCORES], dt.float32, kind="Internal")
    cw_dst = nc.dram_tensor("cw_dst", [E, T // NCORES], dt.float32, kind="Internal")

    with tile.TileContext(nc) as tc:
        with (
            tc.tile_pool(name="const", bufs=1) as cp,
            tc.tile_pool(name="dram", bufs=1, space="DRAM") as dp,
        ):
            ident = cp.tile([P, P], dt.float32)
            make_identity(nc, ident)
            identb = cp.tile([P, P], dt.bfloat16)
            make_identity(nc, identb)
            gwt = cp.tile([P, HC, E], dt.float32)
            nc.sync.dma_start(gwt[:], gwt_in[:])

            ids128 = cp.tile([P, CT], dt.float32)
            cw128 = cp.tile([P, CT], dt.float32)
            idx_i = cp.tile([P, CT], dt.int32)
            idg_i = cp.tile([P, CT], dt.int32)

            # ---------------- routing (1024 local tokens) ----------------
            with (
                tc.tile_pool(name="rt_x", bufs=2) as rx,
                tc.tile_pool(name="rt_misc", bufs=2) as rm,
                tc.tile_pool(name="ps_rt", bufs=1, space="PSUM") as pr,
            ):
                cwA = rm.tile([P, NTLOC, E], dt.float32, tag="cwA", bufs=1)
                for i in range(NTLOC):
                    xt = rx.tile([P, H], dt.float32, tag="xt")
                    nc.sync.dma_start(xt[:], xs_in[i * P : (i + 1) * P, :])
                    ptr = pr.tile([P, HC, P], dt.float32, tag="rt", bufs=2)
                    for hc in range(HC):
                        nc.tensor.transpose(
                            ptr[:, hc, :], xt[:, hc * P : (hc + 1) * P], ident[:]
                        )
                    xT = rm.tile([P, HC, P], dt.float32, tag="rxT")
                    nc.vector.tensor_copy(xT[:], ptr[:])
                    # gate logits in 2 split-K partials (precision: reference
                    # top-2/3 logit gaps go down to ~3e-6; a single 1024-long
                    # fp32 PSUM accumulation chain is too noisy)
                    gp0 = pr.tile([P, E], dt.float32, tag="gp0", bufs=2)
                    gp1 = pr.tile([P, E], dt.float32, tag="gp1", bufs=2)
                    for k, gp in ((0, gp0), (1, gp1)):
                        for s in range(4):
                            nc.tensor.matmul(
                                gp[:], xT[:, 4 * k + s, :], gwt[:, 4 * k + s, :],
                                start=(s == 0), stop=(s == 3),
                            )
                    lg = rm.tile([P, E], dt.float32, tag="lg")
                    nc.vector.tensor_copy(lg[:], gp0[:])
                    nc.vector.tensor_tensor(lg[:], lg[:], gp1[:], op=Alu.add)

                    mx = rm.tile([P, 8], dt.float32, tag="mx")
                    nc.vector.max(mx[:], lg[:])
                    negs = rm.tile([P, 1], dt.float32, tag="negs")
                    nc.vector.tensor_tensor(negs[:], mx[:, 0:1], mx[:, 1:2], op=Alu.add)
                    nc.vector.tensor_scalar_mul(negs[:], negs[:], -1.0)
                    sig = rm.tile([P, E], dt.float32, tag="sig")
                    nc.scalar.activation(sig[:], lg[:], Act.Sigmoid, bias=negs[:], scale=2.0)
                    msk = rm.tile([P, E], dt.float32, tag="msk")
                    nc.vector.tensor_scalar(msk[:], lg[:], mx[:, 1:2], None, op0=Alu.is_ge)
                    nc.vector.tensor_tensor(cwA[:, i, :], sig[:], msk[:], op=Alu.mult)

                # transpose to expert-major so the DRAM staging write is
                # contiguous per partition
                cwT = rm.tile([E, NTLOC, P], dt.float32, tag="cwT", bufs=1)
                with tc.tile_pool(name="ps_cw", bufs=1, space="PSUM") as pw:
                    for i in range(NTLOC):
                        pcw = pw.tile([E, P], dt.float32, tag="pcw", bufs=2)
                        nc.tensor.transpose(pcw[:], cwA[:, i, :], ident[:])
                        nc.vector.tensor_copy(cwT[:, i, :], pcw[:])

                nc.sync.dma_start(
                    cw_src[:].rearrange("e (i p) -> e i p", p=P), cwT[:]
                )

            # ------- all-to-all: cw_dst[s, :] = cw for MY expert, tokens of core s
            nc.gpsimd.collective_compute(
                "AllToAll",
                Alu.bypass,
                replica_groups=[list(range(NCORES))],
                ins=[cw_src[:]],
                outs=[cw_dst[:]],
            )

            # -------- compaction: [8, 1024] buckets (16-channel scatter) --------
            with tc.tile_pool(name="cmp", bufs=1) as sm:
                CH = 16
                cwc = sm.tile([CH, 1024], dt.float32)
                nc.vector.memset(cwc[:], 0.0)
                nc.sync.dma_start(cwc[0:E, :], cw_dst[:])

                mask16 = sm.tile([CH, 1024], dt.float32)
                nc.vector.tensor_scalar(mask16[:], cwc[:], 0.0, None, op0=Alu.is_gt)
                zeros16 = sm.tile([CH, 1024], dt.float32)
                nc.vector.memset(zeros16[:], 0.0)
                scn = sm.tile([CH, 1024], dt.float32)
                nc.vector.tensor_tensor_scan(
                    scn[:], mask16[:], zeros16[:], 0.0, Alu.add, Alu.add
                )
                pos = sm.tile([CH, 1024], dt.float32)
                nc.vector.tensor_tensor(pos[:], scn[:], mask16[:], op=Alu.subtract)
                inb = sm.tile([CH, 1024], dt.float32)
                nc.vector.tensor_scalar(inb[:], pos[:], float(K - 1), None, op0=Alu.is_le)
                sel = sm.tile([CH, 1024], dt.float32)
                nc.vector.tensor_tensor(sel[:], mask16[:], inb[:], op=Alu.mult)
                posf = sm.tile([CH, 1024], dt.float32)
                nc.vector.tensor_tensor(posf[:], pos[:], sel[:], op=Alu.mult)
                selm1 = sm.tile([CH, 1024], dt.float32)
                nc.vector.tensor_scalar(selm1[:], sel[:], 1.0, None, op0=Alu.subtract)
                nc.vector.tensor_tensor(posf[:], posf[:], selm1[:], op=Alu.add)
                posi = sm.tile([CH, 1024], dt.int16)
                nc.vector.tensor_copy(posi[:], posf[:])

                iop1 = sm.tile([CH, 1024], dt.int32)
                nc.gpsimd.iota(iop1[:], pattern=[[1, 1024]], base=1, channel_multiplier=1024)
                idsp1 = sm.tile([CH, 1024], dt.uint16)
                nc.vector.tensor_copy(idsp1[:], iop1[:])

                cwi = cwc[:].bitcast(dt.int32)
                hi_i = sm.tile([CH, 1024], dt.int32)
                nc.vector.tensor_scalar(hi_i[:], cwi, 16, None, op0=Alu.logical_shift_right)
                hi16 = sm.tile([CH, 1024], dt.uint16)
                nc.vector.tensor_copy(hi16[:], hi_i[:])
                lo_i = sm.tile([CH, 1024], dt.int32)
                nc.vector.tensor_scalar(lo_i[:], cwi, 65535, None, op0=Alu.bitwise_and)
                lo16 = sm.tile([CH, 1024], dt.uint16)
                nc.vector.tensor_copy(lo16[:], lo_i[:])

                pc_id = sm.tile([CH, K], dt.uint16)
                nc.gpsimd.local_scatter(pc_id[:], idsp1[:], posi[:], CH, K, 1024)
                pc_hi = sm.tile([CH, K], dt.uint16)
                nc.gpsimd.local_scatter(pc_hi[:], hi16[:], posi[:], CH, K, 1024)
                pc_lo = sm.tile([CH, K], dt.uint16)
                nc.gpsimd.local_scatter(pc_lo[:], lo16[:], posi[:], CH, K, 1024)

                hiK = sm.tile([CH, K], dt.int32)
                nc.vector.tensor_copy(hiK[:], pc_hi[:])
                nc.vector.tensor_scalar(hiK[:], hiK[:], 16, None, op0=Alu.logical_shift_left)
                loK = sm.tile([CH, K], dt.int32)
                nc.vector.tensor_copy(loK[:], pc_lo[:])
                cwK = sm.tile([CH, K], dt.int32)
                nc.vector.tensor_tensor(cwK[:], hiK[:], loK[:], op=Alu.bitwise_or)

                idfK = sm.tile([CH, K], dt.float32)
                nc.vector.tensor_copy(idfK[:], pc_id[:])
                zt = sm.tile([CH, K], dt.float32)
                nc.vector.tensor_scalar(
                    zt[:], idfK[:], 0.0, float(T + 1), op0=Alu.is_equal, op1=Alu.mult
                )
                nc.vector.tensor_tensor(idfK[:], idfK[:], zt[:], op=Alu.add)
                nc.vector.tensor_scalar(idfK[:], idfK[:], 1.0, None, op0=Alu.subtract)

                idxflat = dp.tile([C], dt.float32)
                nc.sync.dma_start(
                    idxflat[:].rearrange("(p f) -> p f", p=E), idfK[0:E, :]
                )
                cwflat2 = dp.tile([C], dt.float32)
                nc.sync.dma_start(
                    cwflat2[:].rearrange("(p f) -> p f", p=E),
                    cwK[0:E, :].bitcast(dt.float32),
                )

                nc.sync.dma_start(ids128[:], idxflat[:].rearrange("(j p) -> p j", p=P))
                nc.sync.dma_start(cw128[:], cwflat2[:].rearrange("(j p) -> p j", p=P))

                nc.vector.tensor_copy(idx_i[:], ids128[:])
                nc.sync.dma_start(idx_out[:].rearrange("(j p) -> p j", p=P), idx_i[:])
                idg_f = sm.tile([P, CT], dt.float32)
                nc.vector.tensor_scalar_min(idg_f[:], ids128[:], float(T - 1))
                nc.vector.tensor_copy(idg_i[:], idg_f[:])

            # ---------------- expert FFN on compact tokens ----------------
            with (
                tc.tile_pool(name="f_act", bufs=1) as fa,
                tc.tile_pool(name="f_gx", bufs=2) as fgx,
                tc.tile_pool(name="f_w", bufs=2) as fw,
                tc.tile_pool(name="f_misc", bufs=2) as fm,
            ):
                xTc = fa.tile([P, HC, C], dt.bfloat16, tag="xTc")
                # gather + transpose the compacted token rows (bf16)
                with tc.tile_pool(name="ps_tr", bufs=1, space="PSUM") as pt:
                    for j in range(CT):
                        gx = fgx.tile([P, H], dt.bfloat16, tag="gx")
                        nc.gpsimd.indirect_dma_start(
                            out=gx[:],
                            out_offset=None,
                            in_=xb_in[:],
                            in_offset=IndirectOffsetOnAxis(
                                ap=idg_i[:, j : j + 1], axis=0
                            ),
                        )
                        ptf = pt.tile([P, HC, P], dt.bfloat16, tag="ft", bufs=2)
                        for hc in range(HC):
                            nc.tensor.transpose(
                                ptf[:, hc, :], gx[:, hc * P : (hc + 1) * P], identb[:]
                            )
                        nc.vector.tensor_copy(xTc[:, :, j * P : (j + 1) * P], ptf[:])

                hT = fa.tile([P, FTP, C], dt.bfloat16, tag="hT")
                w2all = fa.tile([P, FTP, H], dt.bfloat16, tag="w2all")
                with tc.tile_pool(name="ps_ffn", bufs=1, space="PSUM") as pf:
                    for ps_ in range(2):
                        # ---- h = silu(x@w1) * (x@w3) for this f-half ----
                        for ftl in range(FTP):
                            ft = ps_ * FTP + ftl
                            w1s = fw.tile([P, HC, P], dt.bfloat16, tag="w1")
                            nc.sync.dma_start(w1s[:], w1_in[ft])
                            w3s = fw.tile([P, HC, P], dt.bfloat16, tag="w3")
                            nc.scalar.dma_start(w3s[:], w3_in[ft])
                            nc.gpsimd.dma_start(w2all[:, ftl, :], w2_in[ft])
                            s0 = 0
                            for ns in SUBS:
                                pa = pf.tile([P, 512], dt.float32, tag="pa", bufs=2)
                                pb = pf.tile([P, 512], dt.float32, tag="pb", bufs=2)
                                for hc in range(HC):
                                    nc.tensor.matmul(
                                        pa[:, :ns], w1s[:, hc, :],
                                        xTc[:, hc, s0 : s0 + ns],
                                        start=(hc == 0), stop=(hc == HC - 1),
                                    )
                                for hc in range(HC):
                                    nc.tensor.matmul(
                                        pb[:, :ns], w3s[:, hc, :],
                                        xTc[:, hc, s0 : s0 + ns],
                                        start=(hc == 0), stop=(hc == HC - 1),
                                    )
                                sl = fm.tile([P, 512], dt.float32, tag="sl")
                                nc.scalar.activation(sl[:, :ns], pa[:, :ns], Act.Silu)
                                nc.vector.tensor_tensor(
                                    hT[:, ftl, s0 : s0 + ns], sl[:, :ns], pb[:, :ns],
                                    op=Alu.mult,
                                )
                                s0 += ns

                        # ---- y(pass) = h @ w2(f-half), PSUM-accumulated ----
                        for j in range(CT):
                            for hf in range(2):
                                py = pf.tile([P, 512], dt.float32, tag="py", bufs=2)
                                for ftl in range(FTP):
                                    nc.tensor.matmul(
                                        py[:],
                                        hT[:, ftl, j * P : (j + 1) * P],
                                        w2all[:, ftl, hf * 512 : (hf + 1) * 512],
                                        start=(ftl == 0), stop=(ftl == FTP - 1),
                                    )
                                ysb = fm.tile([P, 512], dt.float32, tag="ysb")
                                nc.vector.tensor_scalar(
                                    ysb[:], py[:], cw128[:, j : j + 1], None,
                                    op0=Alu.mult,
                                )
                                nc.sync.dma_start(
                                    y_out[ps_].rearrange("(a p) h -> p a h", p=P)[
                                        :, j, hf * 512 : (hf + 1) * 512
                                    ],
                                    ysb[:],
                                )

    nc.finalize()
    return nc


def _prep_weights(gate_w, w1, w2, w3, e):
    gwt = np.ascontiguousarray(
        gate_w.T.reshape(HC, P, E).transpose(1, 0, 2)
    ).astype(np.float32)
    w1t = np.ascontiguousarray(
        w1[e].reshape(HC, P, FT, P).transpose(2, 1, 0, 3)
    ).astype(BF16)
    w3t = np.ascontiguousarray(
        w3[e].reshape(HC, P, FT, P).transpose(2, 1, 0, 3)
    ).astype(BF16)
    w2t = np.ascontiguousarray(w2[e].reshape(FT, P, H)).astype(BF16)
    return {"gwt": gwt, "w1t": w1t, "w3t": w3t, "w2t": w2t}


def _run(inputs, trace=False):
    from concourse.bass_utils import run_bass_kernel_spmd

    x = np.ascontiguousarray(np.asarray(inputs["x"], dtype=np.float32))
    gate_w = np.ascontiguousarray(np.asarray(inputs["gate_w"], dtype=np.float32))
    w1 = np.asarray(inputs["w1"], dtype=np.float32)
    w2 = np.asarray(inputs["w2"], dtype=np.float32)
    w3 = np.asarray(inputs["w3"], dtype=np.float32)
    xf = x.reshape(T, H)
    xb = xf.astype(BF16)

    # capacity safety check (host-side routing estimate; K has margin over
    # the boundary-rounding uncertainty of this estimate)
    logits = xf @ gate_w.T
    m2 = np.sort(logits, axis=1)[:, -2:-1]
    mask = logits >= m2
    pp = mask.reshape(E, T // E, E).sum(axis=1)
    if pp.max() > K - 2:
        raise RuntimeError(
            f"per-bucket expert token count {pp.max()} exceeds compiled "
            f"capacity K={K}; rebuild kernel.py with a larger K"
        )

    if "nc" not in _cache:
        _cache["nc"] = _build_nc()
    nc = _cache["nc"]

    TS = T // NCORES
    in_maps = []
    for e in range(NCORES):
        m = dict(_prep_weights(gate_w, w1, w2, w3, e))
        m["xs"] = np.ascontiguousarray(xf[e * TS : (e + 1) * TS])
        m["xb"] = xb
        in_maps.append(m)
    res = run_bass_kernel_spmd(nc, in_maps, core_ids=list(range(NCORES)), trace=trace)

    out = np.zeros((T + 1, H), dtype=np.float32)
    for e in range(NCORES):
        idx = res.results[e]["idx"]
        y = res.results[e]["y"]
        out[idx] += y[0] + y[1]
    return out[:T].reshape(x.shape), res


def kernel(**inputs) -> np.ndarray:
    out, _ = _run(inputs, trace=False)
    return out


# revision 7
# speedup vs baseline: 1.2814x; 1.2814x over previous
"""MoE layer (top-2 of 8 experts) on 8 Trainium2 NeuronCores, expert-parallel.

v2 vs baseline:
- Routing distributed: each core routes only its 1024-token slice (8 tiles),
  computes combine-weights for all 8 experts, and an on-device AllToAll
  exchanges them so core e ends up with cw[:, e] for all 8192 tokens.
- FFN in bf16 (x pre-cast on host, weights pre-cast bf16 on host): bf16
  transposes (1 cyc/row vs 2), halved DMA traffic, and weights streamed
  exactly once per run (baseline re-streamed all 50MB fp32 5x).
- Capacity 2560 -> 2176 compact slots: [8 x 1024]-bucket compaction (K=288
  vs actual seed max 282) followed by a cross-bucket packing pass that lands
  each bucket at its exclusive-prefix offset via dynamic-offset DMAs; the FFN
  runs 2 f-half passes so hT fits SBUF.
- w2 contraction accumulated in PSUM (16 matmuls/bank) instead of DVE adds.

Routing numerics (fp32 transposes + split-K fp32r gate matmuls + sigmoid
renorm) are bit-identical to the baseline: min top2/top3 logit gap is 2.8e-6
so the top-2 selection must match the reference exactly.

Self-contained: hardcodes shapes x[4,2048,1024], 8 experts, H=1024, F=4096.
"""

import os

os.environ.setdefault("JAX_PLATFORMS", "")

import numpy as np
import ml_dtypes

BF16 = ml_dtypes.bfloat16

T, H, F, E = 8192, 1024, 4096, 8
P = 128
NCORES = 8
K = 288                  # stage-1 per-bucket slot capacity ([8, 1024] buckets)
C = 2176                 # compact slots per expert after cross-bucket packing
CT = C // P              # 17 slot tiles
SUBS = [512, 512, 512, 512, 128]
assert sum(SUBS) == C
NTLOC = (T // NCORES) // P   # 8 routing tiles per core
HC = H // P              # 8 h-blocks
FT = F // P              # 32 f-blocks
FTP = FT // 2            # 16 f-blocks per pass

_cache: dict = {}


def _build_nc():
    import concourse.mybir as mybir
    import concourse.tile as tile
    from concourse import bacc
    from concourse.bass import IndirectOffsetOnAxis, ds
    from concourse.masks import make_identity

    dt = mybir.dt
    Alu = mybir.AluOpType
    Act = mybir.ActivationFunctionType

    nc = bacc.Bacc("TRN2", target_bir_lowering=False, num_devices=NCORES)

    xs_in = nc.dram_tensor("xs", [T // NCORES, H], dt.float32, kind="ExternalInput")
    xb_in = nc.dram_tensor("xb", [T, H], dt.bfloat16, kind="ExternalInput")
    gwt_in = nc.dram_tensor("gwt", [P, HC, E], dt.float32, kind="ExternalInput")
    w1_in = nc.dram_tensor("w1t", [FT, P, HC, P], dt.bfloat16, kind="ExternalInput")
    w3_in = nc.dram_tensor("w3t", [FT, P, HC, P], dt.bfloat16, kind="ExternalInput")
    w2_in = nc.dram_tensor("w2t", [FT, P, H], dt.bfloat16, kind="ExternalInput")

    y_out = nc.dram_tensor("y", [2, C, H], dt.float32, kind="ExternalOutput")
    idx_out = nc.dram_tensor("idx", [C], dt.int32, kind="ExternalOutput")

    cw_src = nc.dram_tensor("cw_src", [E, T // NCORES], dt.float32, kind="Internal")
    cw_dst = nc.dram_tensor("cw_dst", [E, T // NCORES], dt.float32, kind="Internal")

    with tile.TileContext(nc) as tc:
        with (
            tc.tile_pool(name="const", bufs=1) as cp,
            tc.tile_pool(name="dram", bufs=1, space="DRAM") as dp,
        ):
            ident = cp.tile([P, P], dt.float32)
            make_identity(nc, ident)
            identb = cp.tile([P, P], dt.bfloat16)
            make_identity(nc, identb)
            gwt = cp.tile([P, HC, E], dt.float32)
            nc.sync.dma_start(gwt[:], gwt_in[:])

            ids128 = cp.tile([P, CT], dt.float32)
            cw128 = cp.tile([P, CT], dt.float32)
            idx_i = cp.tile([P, CT], dt.int32)
            idg_i = cp.tile([P, CT], dt.int32)

            # ---------------- routing (1024 local tokens) ----------------
            with (
                tc.tile_pool(name="rt_x", bufs=2) as rx,
                tc.tile_pool(name="rt_misc", bufs=2) as rm,
                tc.tile_pool(name="ps_rt", bufs=1, space="PSUM") as pr,
            ):
                cwA = rm.tile([P, NTLOC, E], dt.float32, tag="cwA", bufs=1)
                for i in range(NTLOC):
                    xt = rx.tile([P, H], dt.float32, tag="xt")
                    nc.sync.dma_start(xt[:], xs_in[i * P : (i + 1) * P, :])
                    ptr = pr.tile([P, HC, P], dt.float32, tag="rt", bufs=2)
                    for hc in range(HC):
                        nc.tensor.transpose(
                            ptr[:, hc, :], xt[:, hc * P : (hc + 1) * P], ident[:]
                        )
                    xT = rm.tile([P, HC, P], dt.float32, tag="rxT")
                    nc.vector.tensor_copy(xT[:], ptr[:])
                    # gate logits in 2 split-K partials (precision: reference
                    # top-2/3 logit gaps go down to ~3e-6; a single 1024-long
                    # fp32 PSUM accumulation chain is too noisy)
                    gp0 = pr.tile([P, E], dt.float32, tag="gp0", bufs=2)
                    gp1 = pr.tile([P, E], dt.float32, tag="gp1", bufs=2)
                    for k, gp in ((0, gp0), (1, gp1)):
                        for s in range(4):
                            nc.tensor.matmul(
                                gp[:], xT[:, 4 * k + s, :], gwt[:, 4 * k + s, :],
                                start=(s == 0), stop=(s == 3),
                            )
                    lg = rm.tile([P, E], dt.float32, tag="lg")
                    nc.vector.tensor_copy(lg[:], gp0[:])
                    nc.vector.tensor_tensor(lg[:], lg[:], gp1[:], op=Alu.add)

                    mx = rm.tile([P, 8], dt.float32, tag="mx")
                    nc.vector.max(mx[:], lg[:])
                    negs = rm.tile([P, 1], dt.float32, tag="negs")
                    nc.vector.tensor_tensor(negs[:], mx[:, 0:1], mx[:, 1:2], op=Alu.add)
                    nc.vector.tensor_scalar_mul(negs[:], negs[:], -1.0)
                    sig = rm.tile([P, E], dt.float32, tag="sig")
                    nc.scalar.activation(sig[:], lg[:], Act.Sigmoid, bias=negs[:], scale=2.0)
                    msk = rm.tile([P, E], dt.float32, tag="msk")
                    nc.vector.tensor_scalar(msk[:], lg[:], mx[:, 1:2], None, op0=Alu.is_ge)
                    nc.vector.tensor_tensor(cwA[:, i, :], sig[:], msk[:], op=Alu.mult)

                # transpose to expert-major so the DRAM staging write is
                # contiguous per partition
                cwT = rm.tile([E, NTLOC, P], dt.float32, tag="cwT", bufs=1)
                with tc.tile_pool(name="ps_cw", bufs=1, space="PSUM") as pw:
                    for i in range(NTLOC):
                        pcw = pw.tile([E, P], dt.float32, tag="pcw", bufs=2)
                        nc.tensor.transpose(pcw[:], cwA[:, i, :], ident[:])
                        nc.vector.tensor_copy(cwT[:, i, :], pcw[:])

                nc.sync.dma_start(
                    cw_src[:].rearrange("e (i p) -> e i p", p=P), cwT[:]
                )

            # ------- all-to-all: cw_dst[s, :] = cw for MY expert, tokens of core s
            nc.gpsimd.collective_compute(
                "AllToAll",
                Alu.bypass,
                replica_groups=[list(range(NCORES))],
                ins=[cw_src[:]],
                outs=[cw_dst[:]],
            )

            # -------- compaction: [8, 1024] buckets (16-channel scatter) --------
            with tc.tile_pool(name="cmp", bufs=1) as sm:
                CH = 16
                cwc = sm.tile([CH, 1024], dt.float32)
                nc.vector.memset(cwc[:], 0.0)
                nc.sync.dma_start(cwc[0:E, :], cw_dst[:])

                mask16 = sm.tile([CH, 1024], dt.float32)
                nc.vector.tensor_scalar(mask16[:], cwc[:], 0.0, None, op0=Alu.is_gt)
                zeros16 = sm.tile([CH, 1024], dt.float32)
                nc.vector.memset(zeros16[:], 0.0)
                scn = sm.tile([CH, 1024], dt.float32)
                nc.vector.tensor_tensor_scan(
                    scn[:], mask16[:], zeros16[:], 0.0, Alu.add, Alu.add
                )
                pos = sm.tile([CH, 1024], dt.float32)
                nc.vector.tensor_tensor(pos[:], scn[:], mask16[:], op=Alu.subtract)
                inb = sm.tile([CH, 1024], dt.float32)
                nc.vector.tensor_scalar(inb[:], pos[:], float(K - 1), None, op0=Alu.is_le)
                sel = sm.tile([CH, 1024], dt.float32)
                nc.vector.tensor_tensor(sel[:], mask16[:], inb[:], op=Alu.mult)
                posf = sm.tile([CH, 1024], dt.float32)
                nc.vector.tensor_tensor(posf[:], pos[:], sel[:], op=Alu.mult)
                selm1 = sm.tile([CH, 1024], dt.float32)
                nc.vector.tensor_scalar(selm1[:], sel[:], 1.0, None, op0=Alu.subtract)
                nc.vector.tensor_tensor(posf[:], posf[:], selm1[:], op=Alu.add)
                posi = sm.tile([CH, 1024], dt.int16)
                nc.vector.tensor_copy(posi[:], posf[:])

                iop1 = sm.tile([CH, 1024], dt.int32)
                nc.gpsimd.iota(iop1[:], pattern=[[1, 1024]], base=1, channel_multiplier=1024)
                idsp1 = sm.tile([CH, 1024], dt.uint16)
                nc.vector.tensor_copy(idsp1[:], iop1[:])

                cwi = cwc[:].bitcast(dt.int32)
                hi_i = sm.tile([CH, 1024], dt.int32)
                nc.vector.tensor_scalar(hi_i[:], cwi, 16, None, op0=Alu.logical_shift_right)
                hi16 = sm.tile([CH, 1024], dt.uint16)
                nc.vector.tensor_copy(hi16[:], hi_i[:])
                lo_i = sm.tile([CH, 1024], dt.int32)
                nc.vector.tensor_scalar(lo_i[:], cwi, 65535, None, op0=Alu.bitwise_and)
                lo16 = sm.tile([CH, 1024], dt.uint16)
                nc.vector.tensor_copy(lo16[:], lo_i[:])

                pc_id = sm.tile([CH, K], dt.uint16)
                nc.gpsimd.local_scatter(pc_id[:], idsp1[:], posi[:], CH, K, 1024)
                pc_hi = sm.tile([CH, K], dt.uint16)
                nc.gpsimd.local_scatter(pc_hi[:], hi16[:], posi[:], CH, K, 1024)
                pc_lo = sm.tile([CH, K], dt.uint16)
                nc.gpsimd.local_scatter(pc_lo[:], lo16[:], posi[:], CH, K, 1024)

                hiK = sm.tile([CH, K], dt.int32)
                nc.vector.tensor_copy(hiK[:], pc_hi[:])
                nc.vector.tensor_scalar(hiK[:], hiK[:], 16, None, op0=Alu.logical_shift_left)
                loK = sm.tile([CH, K], dt.int32)
                nc.vector.tensor_copy(loK[:], pc_lo[:])
                cwK = sm.tile([CH, K], dt.int32)
                nc.vector.tensor_tensor(cwK[:], hiK[:], loK[:], op=Alu.bitwise_or)

                idfK = sm.tile([CH, K], dt.float32)
                nc.vector.tensor_copy(idfK[:], pc_id[:])
                zt = sm.tile([CH, K], dt.float32)
                nc.vector.tensor_scalar(
                    zt[:], idfK[:], 0.0, float(T + 1), op0=Alu.is_equal, op1=Alu.mult
                )
                nc.vector.tensor_tensor(idfK[:], idfK[:], zt[:], op=Alu.add)
                nc.vector.tensor_scalar(idfK[:], idfK[:], 1.0, None, op0=Alu.subtract)

                # ---- cross-bucket packing: bucket b lands at its exclusive
                # prefix offset, so slots [0, total) are densely packed and the
                # FFN only needs ceil(2176/128)=17 slot tiles. Ascending issue
                # order makes bucket b+1's real head overwrite bucket b's
                # padding tail; only adjacent buckets overlap (n_b+n_{b+1} >=
                # 2*246 > K). The staging has K slack for bucket 7's tail.
                cnt1 = sm.tile([1, CH], dt.float32)
                with tc.tile_pool(name="ps_cnt", bufs=1, space="PSUM") as pq:
                    pcnt = pq.tile([1, CH], dt.float32)
                    nc.tensor.transpose(
                        pcnt[:], scn[:, 1023:1024], ident[0:CH, 0:CH]
                    )
                    nc.vector.tensor_copy(cnt1[:], pcnt[:])
                zrow = sm.tile([1, CH], dt.float32)
                nc.vector.memset(zrow[:], 0.0)
                incl = sm.tile([1, CH], dt.float32)
                nc.vector.tensor_tensor_scan(
                    incl[:], cnt1[:], zrow[:], 0.0, Alu.add, Alu.add
                )
                excl = sm.tile([1, CH], dt.float32)
                nc.vector.tensor_tensor(excl[:], incl[:], cnt1[:], op=Alu.subtract)
                offi = sm.tile([1, CH], dt.int32)
                nc.vector.tensor_copy(offi[:], excl[:])

                fill_id = sm.tile([P, CT], dt.float32)
                nc.vector.memset(fill_id[:], float(T))
                fill_cw = sm.tile([P, CT], dt.float32)
                nc.vector.memset(fill_cw[:], 0.0)
                idxflat = dp.tile([C + K], dt.float32)
                nc.sync.dma_start(
                    idxflat[0:C].rearrange("(j p) -> p j", p=P), fill_id[:]
                )
                cwflat2 = dp.tile([C + K], dt.float32)
                nc.scalar.dma_start(
                    cwflat2[0:C].rearrange("(j p) -> p j", p=P), fill_cw[:]
                )
                eng_sp_act = [mybir.EngineType.SP, mybir.EngineType.Activation]
                for b in range(E):
                    off = nc.values_load(
                        offi[0:1, b : b + 1], engines=eng_sp_act,
                        min_val=0, max_val=C,
                    )
                    nc.sync.dma_start(idxflat[ds(off, K)], idfK[b : b + 1, :])
                    nc.scalar.dma_start(
                        cwflat2[ds(off, K)], cwK[b : b + 1, :].bitcast(dt.float32)
                    )

                nc.sync.dma_start(
                    ids128[:], idxflat[0:C].rearrange("(j p) -> p j", p=P)
                )
                nc.sync.dma_start(
                    cw128[:], cwflat2[0:C].rearrange("(j p) -> p j", p=P)
                )

                nc.vector.tensor_copy(idx_i[:], ids128[:])
                nc.sync.dma_start(idx_out[:].rearrange("(j p) -> p j", p=P), idx_i[:])
                idg_f = sm.tile([P, CT], dt.float32)
                nc.vector.tensor_scalar_min(idg_f[:], ids128[:], float(T - 1))
                nc.vector.tensor_copy(idg_i[:], idg_f[:])

            # ---------------- expert FFN on compact tokens ----------------
            with (
                tc.tile_pool(name="f_act", bufs=1) as fa,
                tc.tile_pool(name="f_gx", bufs=4) as fgx,
                tc.tile_pool(name="f_w", bufs=4) as fw,
                tc.tile_pool(name="f_misc", bufs=3) as fm,
            ):
                xTc = fa.tile([P, HC, C], dt.bfloat16, tag="xTc")
                # gather + transpose the compacted token rows (bf16)
                with tc.tile_pool(name="ps_tr", bufs=1, space="PSUM") as pt:
                    for j in range(CT):
                        gx = fgx.tile([P, H], dt.bfloat16, tag="gx")
                        nc.gpsimd.indirect_dma_start(
                            out=gx[:],
                            out_offset=None,
                            in_=xb_in[:],
                            in_offset=IndirectOffsetOnAxis(
                                ap=idg_i[:, j : j + 1], axis=0
                            ),
                        )
                        ptf = pt.tile([P, HC, P], dt.bfloat16, tag="ft", bufs=2)
                        for hc in range(HC):
                            nc.tensor.transpose(
                                ptf[:, hc, :], gx[:, hc * P : (hc + 1) * P], identb[:]
                            )
                        nc.vector.tensor_copy(xTc[:, :, j * P : (j + 1) * P], ptf[:])

                hT = fa.tile([P, FTP, C], dt.bfloat16, tag="hT")
                w2all = fa.tile([P, FTP, H], dt.bfloat16, tag="w2all")
                with tc.tile_pool(name="ps_ffn", bufs=1, space="PSUM") as pf:
                    for ps_ in range(2):
                        # ---- h = silu(x@w1) * (x@w3) for this f-half ----
                        for ftl in range(FTP):
                            ft = ps_ * FTP + ftl
                            w1s = fw.tile([P, HC, P], dt.bfloat16, tag="w1")
                            nc.sync.dma_start(w1s[:], w1_in[ft])
                            w3s = fw.tile([P, HC, P], dt.bfloat16, tag="w3")
                            nc.scalar.dma_start(w3s[:], w3_in[ft])
                            nc.gpsimd.dma_start(w2all[:, ftl, :], w2_in[ft])
                            s0 = 0
                            for ns in SUBS:
                                pa = pf.tile([P, 512], dt.float32, tag="pa", bufs=2)
                                pb = pf.tile([P, 512], dt.float32, tag="pb", bufs=2)
                                for hc in range(HC):
                                    nc.tensor.matmul(
                                        pa[:, :ns], w1s[:, hc, :],
                                        xTc[:, hc, s0 : s0 + ns],
                                        start=(hc == 0), stop=(hc == HC - 1),
                                    )
                                for hc in range(HC):
                                    nc.tensor.matmul(
                                        pb[:, :ns], w3s[:, hc, :],
                                        xTc[:, hc, s0 : s0 + ns],
                                        start=(hc == 0), stop=(hc == HC - 1),
                                    )
                                sl = fm.tile([P, 512], dt.float32, tag="sl")
                                nc.scalar.activation(sl[:, :ns], pa[:, :ns], Act.Silu)
                                nc.vector.tensor_tensor(
                                    hT[:, ftl, s0 : s0 + ns], sl[:, :ns], pb[:, :ns],
                                    op=Alu.mult,
                                )
                                s0 += ns

                        # ---- y(pass) = h @ w2(f-half), PSUM-accumulated ----
                        for j in range(CT):
                            for hf in range(2):
                                py = pf.tile([P, 512], dt.float32, tag="py", bufs=2)
                                for ftl in range(FTP):
                                    nc.tensor.matmul(
                                        py[:],
                                        hT[:, ftl, j * P : (j + 1) * P],
                                        w2all[:, ftl, hf * 512 : (hf + 1) * 512],
                                        start=(ftl == 0), stop=(ftl == FTP - 1),
                                    )
                                ysb = fm.tile([P, 512], dt.float32, tag="ysb")
                                nc.vector.tensor_scalar(
                                    ysb[:], py[:], cw128[:, j : j + 1], None,
                                    op0=Alu.mult,
                                )
                                nc.sync.dma_start(
                                    y_out[ps_].rearrange("(a p) h -> p a h", p=P)[
                                        :, j, hf * 512 : (hf + 1) * 512
                                    ],
                                    ysb[:],
                                )

    nc.finalize()
    return nc


def _prep_weights(gate_w, w1, w2, w3, e):
    gwt = np.ascontiguousarray(
        gate_w.T.reshape(HC, P, E).transpose(1, 0, 2)
    ).astype(np.float32)
    w1t = np.ascontiguousarray(
        w1[e].reshape(HC, P, FT, P).transpose(2, 1, 0, 3)
    ).astype(BF16)
    w3t = np.ascontiguousarray(
        w3[e].reshape(HC, P, FT, P).transpose(2, 1, 0, 3)
    ).astype(BF16)
    w2t = np.ascontiguousarray(w2[e].reshape(FT, P, H)).astype(BF16)
    return {"gwt": gwt, "w1t": w1t, "w3t": w3t, "w2t": w2t}


def _run(inputs, trace=False):
    from concourse.bass_utils import run_bass_kernel_spmd

    x = np.ascontiguousarray(np.asarray(inputs["x"], dtype=np.float32))
    gate_w = np.ascontiguousarray(np.asarray(inputs["gate_w"], dtype=np.float32))
    w1 = np.asarray(inputs["w1"], dtype=np.float32)
    w2 = np.asarray(inputs["w2"], dtype=np.float32)
    w3 = np.asarray(inputs["w3"], dtype=np.float32)
    xf = x.reshape(T, H)
    xb = xf.astype(BF16)

    # capacity safety check (host-side routing estimate; K has margin over
    # the boundary-rounding uncertainty of this estimate)
    logits = xf @ gate_w.T
    m2 = np.sort(logits, axis=1)[:, -2:-1]
    mask = logits >= m2
    pp = mask.reshape(E, T // E, E).sum(axis=1)
    if pp.max() > K - 2:
        raise RuntimeError(
            f"per-bucket expert token count {pp.max()} exceeds compiled "
            f"capacity K={K}; rebuild kernel.py with a larger K"
        )
    tot = mask.sum(axis=0)
    if tot.max() > C:
        raise RuntimeError(
            f"per-expert token count {tot.max()} exceeds compiled packed "
            f"capacity C={C}; rebuild kernel.py with a larger C"
        )

    if "nc" not in _cache:
        _cache["nc"] = _build_nc()
    nc = _cache["nc"]

    TS = T // NCORES
    in_maps = []
    for e in range(NCORES):
        m = dict(_prep_weights(gate_w, w1, w2, w3, e))
        m["xs"] = np.ascontiguousarray(xf[e * TS : (e + 1) * TS])
        m["xb"] = xb
        in_maps.append(m)
    res = run_bass_kernel_spmd(nc, in_maps, core_ids=list(range(NCORES)), trace=trace)

    out = np.zeros((T + 1, H), dtype=np.float32)
    for e in range(NCORES):
        idx = res.results[e]["idx"]
        y = res.results[e]["y"]
        out[idx] += y[0] + y[1]
    return out[:T].reshape(x.shape), res


def kernel(**inputs) -> np.ndarray:
    out, _ = _run(inputs, trace=False)
    return out
